# revision 41
# baseline (speedup 1.0000x reference)
"""FFNN-Transducer joint-lattice kernel for 8 Trainium2 NeuronCores.

Data-parallel over batch B=8 (one sample per core). The device computes the
dense T x U1a joint lattice (U1a = max over batch of targets_size+1 -- the
host masks the rest to zero, matching the reference):
    out[t,u,:] = tanh(enc_proj[t,:] + bias[u,:]) @ jw2
where enc_proj = enc @ jw1[:E] is computed on-device and bias[u,:]
(= pred @ jw1[E:] + jb1) comes from the tiny prediction network (host,
<0.3% of FLOPs). jb2 add + ragged masking are host epilogues.

Per-core pipeline, in t-blocks of 128, fp16 everywhere (PSUM fp32):
  DVE:  pre[j, u*128+t] = encprojT[j,t] (bcast over u) + bias_rep[j,u*128+t]
        -- one tensor_add per block; bias_rep is materialized once so both
        innermost dims are step-1 and the op runs in 2x_1P mode (measured
        240 G elem/s). Plus most of the PSUM->SBUF(fp16) evacuation.
  ACT:  hid = tanh(pre), one big instruction per block (1 elem/cycle/lane
        @1.2 GHz), plus a slice of the evacuation for engine balance.
  PE:   prologue encprojT = jw1enc.T @ encT; per (block,u) one
        [128x128]x[128x88] joint matmul, lhsT contiguous -> FWL.
  DMA:  per-block stores [128, U1a*88] fp16, contiguous per partition.

The output leaves the device as int8 [T, U1a*V] with a fixed global scale
(rel-err ~6e-3 vs the 2e-2 gate): 4x less device store + host<->device
transfer than fp32, and the u >= U1a columns are skipped entirely. The
scale multiply is fused into the evacuation instructions at zero cost.
Host dequantizes to fp32, adds jb2, applies the ragged mask.

Measured (neuron-profile, per core): NEFF span ~138 us vs 316 us for the
staged baseline; host<->device bytes 145 MB/call vs 568 MB.
"""

import os
import sys

for _p in ("/opt/trn_rl_repo", "/root/.axon_site/_ro/trn_rl_repo"):
    if os.path.isdir(_p) and _p not in sys.path:
        sys.path.append(_p)

import numpy as np

import concourse.bass as bass
import concourse.tile as tile
from concourse import bacc, mybir
from concourse.bass_utils import run_bass_kernel_spmd

# Problem dims (hardcoded per contract)
B, T, E = 8, 1000, 512
U = 100
U1 = U + 1          # 101 joint positions
H, D, P = 2, 256, 256
J, V = 128, 88
BLANK = V - 1
N_CORES = 8

TB = 128            # t-steps per block
UG = 10             # u-steps per joint PSUM group (1 bank: 5*88*2 cols)
N_ACT_EVAC = 3      # trailing full u-groups per block evacuated on ScalarE

F32 = mybir.dt.float32
F16 = mybir.dt.float16
I8 = mybir.dt.int8

# Output quantization: the lattice output is written as int8 with a global
# scale (supplied at runtime as a per-partition scalar, so no recompile).
# The scale is sized from a stratified host-side sample of the lattice with
# 1.5x headroom (floor OUT_BOUND_MIN; measured max |expected| = 0.47), so
# int8 rounding costs ~6e-3 rel err against the 2e-2 gate while halving
# the device store + host<->device transfer vs fp16. The scale multiply
# rides the existing evacuation instructions (tensor_scalar /
# activation-scale) at zero extra engine time.
OUT_BOUND_MIN = 0.70
OUT_INT8 = True

_CACHE = {}


def _u_groups(u1a):
    gs = []
    u0 = 0
    while u0 < u1a:
        gs.append((u0, min(UG, u1a - u0)))
        u0 += UG
    return gs


def _build_program(reps=1, u1a=96, nba=8):
    nc = bacc.Bacc("TRN2", target_bir_lowering=False, debug=False)
    tpa = nba * TB

    encT = nc.dram_tensor("encT", [E, tpa], F16, kind="ExternalInput").ap()
    jw1enc = nc.dram_tensor("jw1enc", [E, J], F16, kind="ExternalInput").ap()
    jw2d = nc.dram_tensor("jw2d", [J, V], F16, kind="ExternalInput").ap()
    biasT = nc.dram_tensor("biasT", [J, u1a], F16, kind="ExternalInput").ap()
    oscl = nc.dram_tensor("oscl", [J, 1], F32, kind="ExternalInput").ap()
    out = nc.dram_tensor("out", [T, u1a * V], I8 if OUT_INT8 else F16,
                         kind="ExternalOutput").ap()

    with tile.TileContext(nc) as tc:
        with (
            tc.tile_pool(name="singles", bufs=1) as singles,
            tc.tile_pool(name="prep", bufs=2) as prep,
            tc.tile_pool(name="hidp", bufs=3) as hidp,
            tc.tile_pool(name="stgp", bufs=2) as stgp,
            tc.tile_pool(name="psE", bufs=2, space="PSUM") as psE,
            tc.tile_pool(name="psM", bufs=3, space="PSUM") as psM,
        ):
            # ---- persistent SBUF tensors ----
            # load order matters for the pipeline ramp: biasT first (gates
            # bias_rep -> add0 -> tanh0), big encT tiles last.
            biasT_sb = singles.tile([J, u1a], F16, tag="biasT")
            nc.sync.dma_start(out=biasT_sb[:, :], in_=biasT[:, :])
            oscl_sb = singles.tile([J, 1], F32, tag="oscl")
            nc.sync.dma_start(out=oscl_sb[:, :], in_=oscl[:, :])
            jw1_sb = []
            for k in range(4):
                t_ = singles.tile([128, J], F16, tag=f"jw1_{k}")
                nc.sync.dma_start(out=t_[:, :], in_=jw1enc[k * 128:(k + 1) * 128, :])
                jw1_sb.append(t_)
            jw2_sb = singles.tile([J, V], F16, tag="jw2")
            nc.sync.dma_start(out=jw2_sb[:, :], in_=jw2d[:, :])
            # encT on the scalar HWDGE ring -> overlaps the sync-ring loads
            encT_sb = []
            for k in range(4):
                t_ = singles.tile([128, tpa], F16, tag=f"encT{k}")
                nc.scalar.dma_start(out=t_[:, :], in_=encT[k * 128:(k + 1) * 128, :])
                encT_sb.append(t_)
            bias_rep = singles.tile([J, u1a * TB], F16, tag="biasrep")
            encprojT = singles.tile([J, tpa], F16, tag="encprojT")

            def bias_rep_piece(a, b_):
                # bias_rep[j, u*TB + t] = biasT[j, u]  (2x broadcast copy)
                nc.vector.tensor_copy(
                    out=bias_rep[:, a * TB:b_ * TB]
                    .rearrange("p (u t) -> p u t", u=b_ - a),
                    in_=biasT_sb[:, a:b_].unsqueeze(2)
                    .broadcast_to([J, b_ - a, TB]),
                )

            for rep in range(reps):
                _emit_rep(nc, prep, hidp, stgp, psE, psM,
                          encT_sb, jw1_sb, jw2_sb, bias_rep, encprojT,
                          oscl_sb, out, rep, u1a, nba,
                          bias_rep_piece if rep == 0 else None)

    nc.compile()
    return nc


def _emit_rep(nc, prep, hidp, stgp, psE, psM,
              encT_sb, jw1_sb, jw2_sb, bias_rep, encprojT, oscl_sb,
              out, rep, u1a, nba, bias_rep_piece=None):
    tpa = nba * TB
    groups = _u_groups(u1a)

    def prologue_chunk(idx, c0, c1):
        # encprojT[j, c0:c1] = sum_e jw1enc[e, j] * encT[e, c0:c1]
        ep = psE.tile([J, 512], F32, tag="E", name=f"ep{rep}_{idx}")
        for k in range(4):
            nc.tensor.matmul(
                ep[:, 0:c1 - c0],
                jw1_sb[k][:, :],
                encT_sb[k][:, c0:c1],
                start=(k == 0),
                stop=(k == 3),
            )
        nc.vector.tensor_copy(out=encprojT[:, c0:c1], in_=ep[:, 0:c1 - c0])

    hid_tiles = [None] * nba
    pre_tiles = [None] * nba

    def t_ext(b):
        # last block computes only the t columns that can be stored
        return min(TB, T - b * TB) if b == nba - 1 else TB

    def front_piece(b, a, b_):
        # pre = encprojT (bcast over u) + bias_rep; hid = tanh(pre)
        if hid_tiles[b] is None:
            pre_tiles[b] = prep.tile([128, u1a * TB], F16, tag="pre",
                                     name=f"pre{rep}_{b}")
            hid_tiles[b] = hidp.tile([128, u1a * TB], F16, tag="hid",
                                     name=f"hid{rep}_{b}")
        pre, hid = pre_tiles[b], hid_tiles[b]
        te = t_ext(b)
        nc.vector.tensor_add(
            out=pre[:, a * TB:b_ * TB]
            .rearrange("p (u t) -> p u t", u=b_ - a)[:, :, 0:te],
            in0=(encprojT[:, b * TB:b * TB + te]
                 .unsqueeze(1).broadcast_to([128, b_ - a, te])),
            in1=bias_rep[:, a * TB:b_ * TB]
            .rearrange("p (u t) -> p u t", u=b_ - a)[:, :, 0:te],
        )
        nc.scalar.activation(
            out=hid[:, a * TB:b_ * TB]
            .rearrange("p (u t) -> p u t", u=b_ - a)[:, :, 0:te],
            in_=pre[:, a * TB:b_ * TB]
            .rearrange("p (u t) -> p u t", u=b_ - a)[:, :, 0:te],
            func=mybir.ActivationFunctionType.Tanh,
        )

    def front(b, parts=1):
        step = max(1, u1a // parts)
        bounds = sorted(set(list(range(0, u1a, step)) + [u1a]))
        for a, b_ in zip(bounds[:-1], bounds[1:]):
            front_piece(b, a, b_)

    def evac(dst, src, on_act):
        if OUT_INT8:
            if on_act:
                nc.scalar.activation(out=dst, in_=src,
                                     func=mybir.ActivationFunctionType.Copy,
                                     scale=oscl_sb[:, 0:1])
            else:
                nc.vector.tensor_scalar_mul(out=dst, in0=src,
                                            scalar1=oscl_sb[:, 0:1])
        elif on_act:
            nc.scalar.activation(out=dst, in_=src,
                                 func=mybir.ActivationFunctionType.Copy)
        else:
            nc.vector.tensor_copy(out=dst, in_=src)

    n_full = sum(1 for _, n_u in groups if n_u == UG)

    def back(b):
        # joint matmuls + evacuation (fp32->fp16) + store for block b
        last = b == nba - 1
        last2 = b >= nba - 2
        hid = hid_tiles[b]
        stg = stgp.tile([TB, u1a * V], I8 if OUT_INT8 else F16,
                        tag="stg", name=f"stg{rep}_{b}")
        for g, (u0, n_u) in enumerate(groups):
            M = psM.tile([TB, 1024], F32, tag="M", name=f"M{rep}_{b}_{g}")
            for i in range(n_u):
                col = (i // 5) * 512 + (i % 5) * V
                nc.tensor.matmul(
                    M[:, col:col + V],
                    hid[:, (u0 + i) * TB:(u0 + i + 1) * TB],
                    jw2_sb[:, :],
                    start=True,
                    stop=True,
                )
            if last2:
                # no tanh left to compete with: alternate engines so both
                # drain in lockstep with the PE matmul stream
                on_act = g % 2 == 0
            else:
                on_act = g >= n_full - N_ACT_EVAC and g < n_full
            if n_u == UG:
                evac(stg[:, u0 * V:(u0 + UG) * V]
                     .rearrange("p (bk x) -> p bk x", bk=2),
                     M.rearrange("p (bk x) -> p bk x", bk=2)[:, :, 0:5 * V],
                     on_act)
            elif n_u > 5:
                # ragged tail spanning both banks: two pieces
                evac(stg[:, u0 * V:(u0 + 5) * V], M[:, 0:5 * V], on_act and last2)
                evac(stg[:, (u0 + 5) * V:(u0 + n_u) * V],
                     M[:, 512:512 + (n_u - 5) * V], last2 and not on_act)
            else:
                evac(stg[:, u0 * V:(u0 + n_u) * V], M[:, 0:n_u * V],
                     on_act and last2)
        n_t = min(TB, T - b * TB)
        if n_t > 0:
            nc.sync.dma_start(
                out=out[b * TB:b * TB + n_t, :],
                in_=stg[0:n_t, :],
            )

    # Ramp: interleave bias_rep quarters, the first prologue chunk, and
    # block-0 front quarters so tanh0 starts as early as possible. Blocks
    # 0..3 only need encprojT chunk 0; later chunks are emitted before
    # front(4) comes up in the loop below.
    if bias_rep_piece is not None:
        qs = sorted(set(list(range(0, u1a, max(1, u1a // 4))) + [u1a]))
        bias_rep_piece(qs[0], qs[1])
        prologue_chunk(0, 0, TB)      # mini first chunk: unblocks add0 asap
        front_piece(0, qs[0], qs[1])
        bias_rep_piece(qs[1], qs[2])
        if tpa > TB:
            prologue_chunk(1, TB, min(512, tpa))
        front_piece(0, qs[1], qs[2])
        for a, b_ in zip(qs[2:-1], qs[3:]):
            bias_rep_piece(a, b_)
            front_piece(0, a, b_)
        for i, c0 in enumerate(range(512, tpa, 512)):
            prologue_chunk(2 + i, c0, min(c0 + 512, tpa))
    else:
        for i, c0 in enumerate(range(0, tpa, 512)):
            prologue_chunk(i, c0, min(c0 + 512, tpa))
        front(0, parts=2)
    if nba > 1:
        front(1, parts=2)
    for b in range(nba):
        if b + 2 < nba:
            front(b + 2)
        back(b)


def _host_pred_bias(targets, emb, pw1, pb1, pw2, pb2, jw1, jb1, u1a):
    """bias[b, u, j] = (pred @ jw1[E:] + jb1)[u, j] for all B samples."""
    tgt = np.asarray(targets).astype(np.int64)                # [B, U]
    ext = np.concatenate(
        [np.full((B, H), BLANK, np.int64), tgt], axis=1)      # [B, U+H]
    e = np.concatenate([emb[ext[:, 1:u1a + 1]], emb[ext[:, 0:u1a]]],
                       axis=2).reshape(B, u1a, H * D)         # [B, u1a, 512]
    h = np.tanh(e @ pw1 + pb1)
    pred = np.tanh(h @ pw2 + pb2)
    return pred @ jw1[E:] + jb1                               # [B, u1a, 128]


def _sample_out_bound(encoder_states, bias, jw1, jw2, jb2, u1a, t_max):
    """Stratified 16x16 (t, u) sample of the lattice -> |out| bound."""
    t_idx = np.unique(np.linspace(0, max(t_max - 1, 0), 16).astype(np.int64))
    u_idx = np.unique(np.linspace(0, u1a - 1, 16).astype(np.int64))
    encp = encoder_states[:, t_idx] @ jw1[:E]                 # [B, 16, J]
    hid = np.tanh(encp[:, :, None, :] + bias[:, u_idx][:, None, :, :])
    o = hid @ jw2 + jb2                                       # [B,16,16,V]
    return float(np.abs(o).max()) * 1.5


def _make_in_maps(encoder_states, targets, emb, pw1, pb1, pw2, pb2, jw1, jb1,
                  jw2, jb2, u1a, tpa, t_max):
    encoder_states = np.asarray(encoder_states, dtype=np.float32)
    jw1 = np.asarray(jw1, dtype=np.float32)
    jw2_f = np.ascontiguousarray(np.asarray(jw2, dtype=np.float32))
    jw2_np = jw2_f.astype(np.float16)
    jw1enc = np.ascontiguousarray(jw1[:E]).astype(np.float16)
    bias = _host_pred_bias(
        targets, np.asarray(emb, np.float32),
        np.asarray(pw1, np.float32), np.asarray(pb1, np.float32),
        np.asarray(pw2, np.float32), np.asarray(pb2, np.float32),
        jw1, np.asarray(jb1, np.float32), u1a)               # [B, u1a, J]
    biasT = np.ascontiguousarray(bias.transpose(0, 2, 1)).astype(np.float16)

    bound = max(OUT_BOUND_MIN, _sample_out_bound(
        encoder_states, bias, jw1, jw2_f, np.asarray(jb2, np.float32),
        u1a, t_max))
    oscl = np.full((J, 1), 127.0 / bound, np.float32)

    encT_all = np.zeros((B, E, tpa), np.float16)
    encT_all[:, :, :min(T, tpa)] = \
        encoder_states.transpose(0, 2, 1)[:, :, :min(T, tpa)].astype(np.float16)

    in_maps = []
    for b in range(B):
        in_maps.append({
            "encT": encT_all[b],
            "jw1enc": jw1enc,
            "jw2d": jw2_np,
            "biasT": biasT[b],
            "oscl": oscl,
        })
    return in_maps, bound


def kernel(encoder_states, encoder_states_size, targets, targets_size,
           emb, pw1, pb1, pw2, pb2, jw1, jb1, jw2, jb2):
    tsz = np.asarray(encoder_states_size).astype(np.int64)
    usz = np.asarray(targets_size).astype(np.int64)
    u1a = int(min(U1, usz.max() + 1))
    nba = int(-(-min(T, int(tsz.max())) // TB))
    key = (u1a, nba)
    if _CACHE.get("key") != key:
        _CACHE["nc"] = _build_program(u1a=u1a, nba=nba)
        _CACHE["key"] = key
    nc = _CACHE["nc"]

    in_maps, bound = _make_in_maps(encoder_states, targets, emb, pw1, pb1,
                                   pw2, pb2, jw1, jb1, jw2, jb2, u1a,
                                   nba * TB, int(tsz.max()))
    _CACHE["in_maps"] = in_maps
    res = run_bass_kernel_spmd(nc, in_maps, core_ids=list(range(N_CORES)))

    out = np.zeros((B, T, U1, V), np.float32)
    jb2_f = np.asarray(jb2, np.float32)
    for b in range(B):
        raw = res.results[b]["out"].reshape(T, u1a, V)
        dst = out[b, :, :u1a, :]
        if OUT_INT8:
            np.multiply(raw, np.float32(bound / 127.0), dtype=np.float32,
                        out=dst, casting="unsafe")
            if jb2_f.any():
                dst += jb2_f
        else:
            np.add(raw, jb2_f, dtype=np.float32, out=dst, casting="unsafe")
    # ragged masking (host epilogue)
    for b in range(B):
        out[b, tsz[b]:, :, :] = 0.0
        out[b, :, usz[b] + 1:, :] = 0.0
    return out


# revision 59
# speedup vs baseline: 1.0150x; 1.0150x over previous
"""FFNN-Transducer joint-lattice kernel for 8 Trainium2 NeuronCores.

Data-parallel over batch B=8 (one sample per core). The device computes the
dense T x U1a joint lattice (U1a = max over batch of targets_size+1 -- the
host masks the rest to zero, matching the reference):
    out[t,u,:] = tanh(enc_proj[t,:] + bias[u,:]) @ jw2
where enc_proj = enc @ jw1[:E] is computed on-device and bias[u,:]
(= pred @ jw1[E:] + jb1) comes from the tiny prediction network (host,
<0.3% of FLOPs). jb2 add + ragged masking are host epilogues.

Per-core pipeline, in t-blocks of 128, fp16 everywhere (PSUM fp32):
  DVE:  pre[j, u*128+t] = encprojT[j,t] (bcast over u) + bias_rep[j,u*128+t]
        -- one tensor_add per block; bias_rep is materialized once so both
        innermost dims are step-1 and the op runs in 2x_1P mode (measured
        240 G elem/s). Plus most of the PSUM->SBUF(fp16) evacuation.
  ACT:  hid = tanh(pre), one big instruction per block (1 elem/cycle/lane
        @1.2 GHz), plus a slice of the evacuation for engine balance.
  PE:   prologue encprojT = jw1enc.T @ encT; per (block,u) one
        [128x128]x[128x88] joint matmul, lhsT contiguous -> FWL.
  DMA:  per-block stores [128, U1a*88] fp16, contiguous per partition.

The output leaves the device as int8 [T, U1a*V] with a fixed global scale
(rel-err ~6e-3 vs the 2e-2 gate): 4x less device store + host<->device
transfer than fp32, and the u >= U1a columns are skipped entirely. The
scale multiply is fused into the evacuation instructions at zero cost.
Host dequantizes to fp32, adds jb2, applies the ragged mask.

Measured (neuron-profile, per core): NEFF span ~138 us vs 316 us for the
staged baseline; host<->device bytes 145 MB/call vs 568 MB.
"""

import os
import sys

for _p in ("/opt/trn_rl_repo", "/root/.axon_site/_ro/trn_rl_repo"):
    if os.path.isdir(_p) and _p not in sys.path:
        sys.path.append(_p)

import numpy as np

import concourse.bass as bass
import concourse.tile as tile
from concourse import bacc, mybir
from concourse.bass_utils import run_bass_kernel_spmd

# Problem dims (hardcoded per contract)
B, T, E = 8, 1000, 512
U = 100
U1 = U + 1          # 101 joint positions
H, D, P = 2, 256, 256
J, V = 128, 88
BLANK = V - 1
N_CORES = 8

TB = 128            # t-steps per block
UG = 10             # u-steps per joint PSUM group (1 bank: 5*88*2 cols)
N_ACT_EVAC = 3      # trailing full u-groups per block evacuated on ScalarE

F32 = mybir.dt.float32
F16 = mybir.dt.float16
I8 = mybir.dt.int8

# Output quantization: the lattice output is written as int8 with a global
# scale (supplied at runtime as a per-partition scalar, so no recompile).
# The scale is sized from a stratified host-side sample of the lattice with
# 1.5x headroom (floor OUT_BOUND_MIN; measured max |expected| = 0.47), so
# int8 rounding costs ~6e-3 rel err against the 2e-2 gate while halving
# the device store + host<->device transfer vs fp16. The scale multiply
# rides the existing evacuation instructions (tensor_scalar /
# activation-scale) at zero extra engine time.
OUT_BOUND_MIN = 0.70
OUT_INT8 = True

_CACHE = {}


def _u_groups(u1a):
    gs = []
    u0 = 0
    while u0 < u1a:
        gs.append((u0, min(UG, u1a - u0)))
        u0 += UG
    return gs


def _build_program(reps=1, u1a=96, nba=8):
    nc = bacc.Bacc("TRN2", target_bir_lowering=False, debug=False)
    tpa = nba * TB

    encT = nc.dram_tensor("encT", [E, tpa], F16, kind="ExternalInput").ap()
    jw1enc = nc.dram_tensor("jw1enc", [E, J], F16, kind="ExternalInput").ap()
    jw2d = nc.dram_tensor("jw2d", [J, V], F16, kind="ExternalInput").ap()
    biasT = nc.dram_tensor("biasT", [J, u1a], F16, kind="ExternalInput").ap()
    oscl = nc.dram_tensor("oscl", [J, 1], F32, kind="ExternalInput").ap()
    out = nc.dram_tensor("out", [T, u1a * V], I8 if OUT_INT8 else F16,
                         kind="ExternalOutput").ap()

    with tile.TileContext(nc) as tc:
        with (
            tc.tile_pool(name="singles", bufs=1) as singles,
            tc.tile_pool(name="prep", bufs=2) as prep,
            tc.tile_pool(name="hidp", bufs=3) as hidp,
            tc.tile_pool(name="stgp", bufs=2) as stgp,
            tc.tile_pool(name="psE", bufs=2, space="PSUM") as psE,
            tc.tile_pool(name="psM", bufs=3, space="PSUM") as psM,
        ):
            # ---- persistent SBUF tensors ----
            # load order matters for the pipeline ramp: biasT first (gates
            # bias_rep -> add0 -> tanh0), big encT tiles last.
            biasT_sb = singles.tile([J, u1a], F16, tag="biasT")
            nc.sync.dma_start(out=biasT_sb[:, :], in_=biasT[:, :])
            oscl_sb = singles.tile([J, 1], F32, tag="oscl")
            nc.sync.dma_start(out=oscl_sb[:, :], in_=oscl[:, :])
            jw1_sb = []
            for k in range(4):
                t_ = singles.tile([128, J], F16, tag=f"jw1_{k}")
                nc.sync.dma_start(out=t_[:, :], in_=jw1enc[k * 128:(k + 1) * 128, :])
                jw1_sb.append(t_)
            jw2_sb = singles.tile([J, V], F16, tag="jw2")
            nc.sync.dma_start(out=jw2_sb[:, :], in_=jw2d[:, :])
            # encT on the scalar HWDGE ring -> overlaps the sync-ring loads
            encT_sb = []
            for k in range(4):
                t_ = singles.tile([128, tpa], F16, tag=f"encT{k}")
                nc.scalar.dma_start(out=t_[:, :], in_=encT[k * 128:(k + 1) * 128, :])
                encT_sb.append(t_)
            bias_rep = singles.tile([J, u1a * TB], F16, tag="biasrep")
            encprojT = singles.tile([J, tpa], F16, tag="encprojT")

            def bias_rep_piece(a, b_):
                # bias_rep[j, u*TB + t] = biasT[j, u]  (2x broadcast copy)
                nc.vector.tensor_copy(
                    out=bias_rep[:, a * TB:b_ * TB]
                    .rearrange("p (u t) -> p u t", u=b_ - a),
                    in_=biasT_sb[:, a:b_].unsqueeze(2)
                    .broadcast_to([J, b_ - a, TB]),
                )

            for rep in range(reps):
                _emit_rep(nc, prep, hidp, stgp, psE, psM,
                          encT_sb, jw1_sb, jw2_sb, bias_rep, encprojT,
                          oscl_sb, out, rep, u1a, nba,
                          bias_rep_piece if rep == 0 else None)

    nc.compile()
    return nc


def _emit_rep(nc, prep, hidp, stgp, psE, psM,
              encT_sb, jw1_sb, jw2_sb, bias_rep, encprojT, oscl_sb,
              out, rep, u1a, nba, bias_rep_piece=None):
    tpa = nba * TB
    groups = _u_groups(u1a)

    def prologue_chunk(idx, c0, c1):
        # encprojT[j, c0:c1] = sum_e jw1enc[e, j] * encT[e, c0:c1]
        ep = psE.tile([J, 512], F32, tag="E", name=f"ep{rep}_{idx}")
        for k in range(4):
            nc.tensor.matmul(
                ep[:, 0:c1 - c0],
                jw1_sb[k][:, :],
                encT_sb[k][:, c0:c1],
                start=(k == 0),
                stop=(k == 3),
            )
        nc.vector.tensor_copy(out=encprojT[:, c0:c1], in_=ep[:, 0:c1 - c0])

    hid_tiles = [None] * nba
    pre_tiles = [None] * nba

    def t_ext(b):
        # last block computes only the t columns that can be stored
        return min(TB, T - b * TB) if b == nba - 1 else TB

    def front_piece(b, a, b_):
        # pre = encprojT (bcast over u) + bias_rep; hid = tanh(pre)
        if hid_tiles[b] is None:
            pre_tiles[b] = prep.tile([128, u1a * TB], F16, tag="pre",
                                     name=f"pre{rep}_{b}")
            hid_tiles[b] = hidp.tile([128, u1a * TB], F16, tag="hid",
                                     name=f"hid{rep}_{b}")
        pre, hid = pre_tiles[b], hid_tiles[b]
        te = t_ext(b)
        nc.vector.tensor_add(
            out=pre[:, a * TB:b_ * TB]
            .rearrange("p (u t) -> p u t", u=b_ - a)[:, :, 0:te],
            in0=(encprojT[:, b * TB:b * TB + te]
                 .unsqueeze(1).broadcast_to([128, b_ - a, te])),
            in1=bias_rep[:, a * TB:b_ * TB]
            .rearrange("p (u t) -> p u t", u=b_ - a)[:, :, 0:te],
        )
        nc.scalar.activation(
            out=hid[:, a * TB:b_ * TB]
            .rearrange("p (u t) -> p u t", u=b_ - a)[:, :, 0:te],
            in_=pre[:, a * TB:b_ * TB]
            .rearrange("p (u t) -> p u t", u=b_ - a)[:, :, 0:te],
            func=mybir.ActivationFunctionType.Tanh,
        )

    def front(b, parts=1):
        step = max(1, u1a // parts)
        bounds = sorted(set(list(range(0, u1a, step)) + [u1a]))
        for a, b_ in zip(bounds[:-1], bounds[1:]):
            front_piece(b, a, b_)

    def evac(dst, src, on_act):
        if OUT_INT8:
            if on_act:
                nc.scalar.activation(out=dst, in_=src,
                                     func=mybir.ActivationFunctionType.Copy,
                                     scale=oscl_sb[:, 0:1])
            else:
                nc.vector.tensor_scalar_mul(out=dst, in0=src,
                                            scalar1=oscl_sb[:, 0:1])
        elif on_act:
            nc.scalar.activation(out=dst, in_=src,
                                 func=mybir.ActivationFunctionType.Copy)
        else:
            nc.vector.tensor_copy(out=dst, in_=src)

    n_full = sum(1 for _, n_u in groups if n_u == UG)

    def back(b):
        # joint matmuls + evacuation (fp32->fp16) + store for block b
        last = b == nba - 1
        last2 = b >= nba - 2
        hid = hid_tiles[b]
        stg = stgp.tile([TB, u1a * V], I8 if OUT_INT8 else F16,
                        tag="stg", name=f"stg{rep}_{b}")
        for g, (u0, n_u) in enumerate(groups):
            M = psM.tile([TB, 1024], F32, tag="M", name=f"M{rep}_{b}_{g}")
            for i in range(n_u):
                col = (i // 5) * 512 + (i % 5) * V
                nc.tensor.matmul(
                    M[:, col:col + V],
                    hid[:, (u0 + i) * TB:(u0 + i + 1) * TB],
                    jw2_sb[:, :],
                    start=True,
                    stop=True,
                )
            if last2:
                # no tanh left to compete with: alternate engines so both
                # drain in lockstep with the PE matmul stream
                on_act = g % 2 == 0
            else:
                on_act = g >= n_full - N_ACT_EVAC and g < n_full
            if n_u == UG:
                evac(stg[:, u0 * V:(u0 + UG) * V]
                     .rearrange("p (bk x) -> p bk x", bk=2),
                     M.rearrange("p (bk x) -> p bk x", bk=2)[:, :, 0:5 * V],
                     on_act)
            elif n_u > 5:
                # ragged tail spanning both banks: two pieces
                evac(stg[:, u0 * V:(u0 + 5) * V], M[:, 0:5 * V], on_act and last2)
                evac(stg[:, (u0 + 5) * V:(u0 + n_u) * V],
                     M[:, 512:512 + (n_u - 5) * V], last2 and not on_act)
            else:
                evac(stg[:, u0 * V:(u0 + n_u) * V], M[:, 0:n_u * V],
                     on_act and last2)
        n_t = min(TB, T - b * TB)
        if n_t > 0:
            nc.sync.dma_start(
                out=out[b * TB:b * TB + n_t, :],
                in_=stg[0:n_t, :],
            )

    # Ramp: interleave bias_rep quarters, the first prologue chunk, and
    # block-0 front quarters so tanh0 starts as early as possible. Blocks
    # 0..3 only need encprojT chunk 0; later chunks are emitted before
    # front(4) comes up in the loop below.
    if bias_rep_piece is not None:
        qs = sorted(set(list(range(0, u1a, max(1, u1a // 4))) + [u1a]))
        bias_rep_piece(qs[0], qs[1])
        prologue_chunk(0, 0, TB)      # mini first chunk: unblocks add0 asap
        front_piece(0, qs[0], qs[1])
        bias_rep_piece(qs[1], qs[2])
        if tpa > TB:
            prologue_chunk(1, TB, min(512, tpa))
        front_piece(0, qs[1], qs[2])
        for a, b_ in zip(qs[2:-1], qs[3:]):
            bias_rep_piece(a, b_)
            front_piece(0, a, b_)
        for i, c0 in enumerate(range(512, tpa, 512)):
            prologue_chunk(2 + i, c0, min(c0 + 512, tpa))
    else:
        for i, c0 in enumerate(range(0, tpa, 512)):
            prologue_chunk(i, c0, min(c0 + 512, tpa))
        front(0, parts=2)
    if nba > 1:
        front(1, parts=2)
    for b in range(nba):
        if b + 2 < nba:
            front(b + 2)
        back(b)


def _host_pred_bias(targets, emb, pw1, pb1, pw2, pb2, jw1, jb1, u1a):
    """bias[b, u, j] = (pred @ jw1[E:] + jb1)[u, j] for all B samples."""
    tgt = np.asarray(targets).astype(np.int64)                # [B, U]
    ext = np.concatenate(
        [np.full((B, H), BLANK, np.int64), tgt], axis=1)      # [B, U+H]
    e = np.concatenate([emb[ext[:, 1:u1a + 1]], emb[ext[:, 0:u1a]]],
                       axis=2).reshape(B, u1a, H * D)         # [B, u1a, 512]
    h = np.tanh(e @ pw1 + pb1)
    pred = np.tanh(h @ pw2 + pb2)
    return pred @ jw1[E:] + jb1                               # [B, u1a, 128]


def _sample_out_bound(encoder_states, bias, jw1, jw2, jb2, u1a, t_max):
    """Stratified 16x16 (t, u) sample of the lattice -> |out| bound."""
    t_idx = np.unique(np.linspace(0, max(t_max - 1, 0), 16).astype(np.int64))
    u_idx = np.unique(np.linspace(0, u1a - 1, 16).astype(np.int64))
    encp = encoder_states[:, t_idx] @ jw1[:E]                 # [B, 16, J]
    hid = np.tanh(encp[:, :, None, :] + bias[:, u_idx][:, None, :, :])
    o = hid @ jw2 + jb2                                       # [B,16,16,V]
    return float(np.abs(o).max()) * 1.5


def _make_in_maps(encoder_states, targets, emb, pw1, pb1, pw2, pb2, jw1, jb1,
                  jw2, jb2, u1a, tpa, t_max):
    encoder_states = np.asarray(encoder_states, dtype=np.float32)
    jw1 = np.asarray(jw1, dtype=np.float32)
    jw2_f = np.ascontiguousarray(np.asarray(jw2, dtype=np.float32))
    jw2_np = jw2_f.astype(np.float16)
    jw1enc = np.ascontiguousarray(jw1[:E]).astype(np.float16)
    bias = _host_pred_bias(
        targets, np.asarray(emb, np.float32),
        np.asarray(pw1, np.float32), np.asarray(pb1, np.float32),
        np.asarray(pw2, np.float32), np.asarray(pb2, np.float32),
        jw1, np.asarray(jb1, np.float32), u1a)               # [B, u1a, J]
    biasT = np.ascontiguousarray(bias.transpose(0, 2, 1)).astype(np.float16)

    bound = max(OUT_BOUND_MIN, _sample_out_bound(
        encoder_states, bias, jw1, jw2_f, np.asarray(jb2, np.float32),
        u1a, t_max))
    oscl = np.full((J, 1), 127.0 / bound, np.float32)

    encT_all = np.zeros((B, E, tpa), np.float16)
    encT_all[:, :, :min(T, tpa)] = \
        encoder_states.transpose(0, 2, 1)[:, :, :min(T, tpa)].astype(np.float16)

    in_maps = []
    for b in range(B):
        in_maps.append({
            "encT": encT_all[b],
            "jw1enc": jw1enc,
            "jw2d": jw2_np,
            "biasT": biasT[b],
            "oscl": oscl,
        })
    return in_maps, bound


def kernel(encoder_states, encoder_states_size, targets, targets_size,
           emb, pw1, pb1, pw2, pb2, jw1, jb1, jw2, jb2):
    tsz = np.asarray(encoder_states_size).astype(np.int64)
    usz = np.asarray(targets_size).astype(np.int64)
    u1a = int(min(U1, usz.max() + 1))
    nba = int(-(-min(T, int(tsz.max())) // TB))
    key = (u1a, nba)
    if _CACHE.get("key") != key:
        _CACHE["nc"] = _build_program(u1a=u1a, nba=nba)
        _CACHE["key"] = key
    nc = _CACHE["nc"]

    in_maps, bound = _make_in_maps(encoder_states, targets, emb, pw1, pb1,
                                   pw2, pb2, jw1, jb1, jw2, jb2, u1a,
                                   nba * TB, int(tsz.max()))
    _CACHE["in_maps"] = in_maps
    res = run_bass_kernel_spmd(nc, in_maps, core_ids=list(range(N_CORES)))

    out = np.zeros((B, T, U1, V), np.float32)
    jb2_f = np.asarray(jb2, np.float32)
    for b in range(B):
        raw = res.results[b]["out"].reshape(T, u1a, V)
        dst = out[b, :, :u1a, :]
        if OUT_INT8:
            np.multiply(raw, np.float32(bound / 127.0), dtype=np.float32,
                        out=dst, casting="unsafe")
            if jb2_f.any():
                dst += jb2_f
        else:
            np.add(raw, jb2_f, dtype=np.float32, out=dst, casting="unsafe")
    # ragged masking (host epilogue)
    for b in range(B):
        out[b, tsz[b]:, :, :] = 0.0
        out[b, :, usz[b] + 1:, :] = 0.0
    return out
